# revision 30
# baseline (speedup 1.0000x reference)
"""Trainium2 Bass kernel for CapsuleBlock (dynamic routing).

Reference computation:
  hats[b,n,k,o] = sum_d x[b,n,d] * W[n,k,d,o]       x:[64,2048,8] W:[2048,16,8,16]
  3 routing iterations (softmax over k, weighted sum over n, squash over o)
  output: [64, 16, 16]

Sharding: data-parallel over batch B=64 across 8 cores (B_local=8), W replicated.

Per-core layout notation: n = g*16 + v  (g in [0,128) "group", v in [0,16)),
d in [0,8), k in [0,16) out-capsule, o in [0,16) out-dim.

Key trick: einsum runs on the tensor engine with a block-diagonal lhsT
  lhsT[(v,d), (b,v')] = x[b, g*16+v', d] * delta_{v,v'}
so one 128-wide matmul per group computes hats for 16 n's x 8 b's at once:
  psum[(b,v'), (o,k)] = sum_{(v,d)} lhsT * W[g*16+v, k, d, o]
H lives in SBUF as [p=(b,v'), f=(g, o, k)] in bf16 (8MB). The free dim is
(o,k) — o outer — so the routing's sum-over-o is a 4-level in-place tree of
packed bf16 adds (DVE tensor_reduce gets no 16-bit speedup; packed adds do).

W streaming: W arrives [n,k,d,o]; the matmul needs [(v,d),(o,k)] per group.
Loading that layout directly from HBM costs 1024 DMAs of 64B runs (~10ms).
Instead: 16 x 1MB contiguous loads [n-batch, (k,d,o)], an on-chip permute
copy to [(n),(d,o,k)] in bf16 (1-cycle/col on the PE, and it halves the
reshuffle traffic), then one SBUF->SBUF DMA per group (SBUF-SBUF has no
small-run penalty) into [(v,d),(o,k)].

s0 (uniform-c iteration): gpsimd accumulates 8-group slabs of H as each W
batch completes (off the PE), then a small fp32 tree + ONESB16 matmul
averages over (g, v').

Routing iters 1,2 sweep H per 16-group chunk: bf16 product vs broadcast
OutB + o-tree -> ach; one batched softmax over all groups; c-lhsT built on
gpsimd; 16 accumulating bf16 matmuls per chunk into psum_R1[(b,k'), (o,k)];
diagonal extract (k==k' mask) -> squash -> out.

Toolchain workarounds (this container):
- walrus codegen allows only 1 sync wait/instruction -> run Bacc's
  move_matmul_waits_to_ldweights + generate_event_semaphores passes.
- gpsimd int shift/mod ops and DVE tensor_tensor_reduce hit "ISA wrong
  length" -> constant masks precomputed on host, squash unfused.
- DMA source APs that split the partition dim read wrong partitions ->
  keep rearranges on the DRAM side of every DMA.
"""

import numpy as np

import concourse.bass as bass
import concourse.mybir as mybir
import concourse.tile as tile
from concourse.bass import ds
from concourse.bass_utils import run_bass_kernel_spmd

F32 = mybir.dt.float32
F32R = mybir.dt.float32r
BF16 = mybir.dt.bfloat16
AX = mybir.AxisListType
OP = mybir.AluOpType
ACT_F = mybir.ActivationFunctionType

# per-core problem dims
B = 8        # local batch (64 / 8 cores)
N = 2048     # input capsules
K = 16       # output capsules
O = 16       # output capsule dim
D = 8        # input capsule dim
V = 16       # n's per group
G = N // V   # 128 groups
GL = 8       # groups per W batch
NB = G // GL  # 16 W batches (128 n's each)
P = 128
KO = K * O   # 256

NUM_ROUTINGS = 3
CHUNK = 32   # groups per sweep chunk


# host-precomputed constant masks, packed as one [128, CF] f32 input
# (gpsimd int shift/mod ops hit an "ISA wrong length" walrus codegen bug in
# this toolchain, so the masks cannot be built on-device)
C_IDENT = 0            # [128, 128] identity (PE transpose)
C_M = 128              # [128, B*V]  M[(v,d),(b,v')] = delta_{v,v'}
C_IB = C_M + B * V     # [128, B]    Ib[(b,v'),b'] = delta_{b,b'}
C_IB16 = C_IB + B      # [128, B]    Ib / 16
C_MK = C_IB16 + B      # [128, KO]   MKT[(b,k'),(o,k)] = delta_{k,k'}
CF = C_MK + KO


def _build_consts() -> np.ndarray:
    c = np.zeros((P, CF), dtype=np.float32)
    p = np.arange(P)
    c[:, C_IDENT:C_IDENT + P] = np.eye(P, dtype=np.float32)
    # M: partition p=(v,d) with v=p>>3 ; free f=(b,v') with v'=f%16
    f = np.arange(B * V)
    c[:, C_M:C_M + B * V] = ((p >> 3)[:, None] == (f % V)[None, :])
    # Ib: partition p=(b,v') with b=p>>4 ; free b'
    fb = np.arange(B)
    c[:, C_IB:C_IB + B] = ((p >> 4)[:, None] == fb[None, :])
    c[:, C_IB16:C_IB16 + B] = c[:, C_IB:C_IB + B] / V
    # MKT: partition p=(b,k') with k'=p%16 ; free f=(o,k) with k=f%16
    fk = np.arange(KO)
    c[:, C_MK:C_MK + KO] = ((p % K)[:, None] == (fk % K)[None, :])
    return c


def build_kernel(reps=1, n_sweeps=NUM_ROUTINGS - 1, swdge_queues=4,
                 wt_queue="sync", chunk=None):
    nc = bass.Bass(trn_type="TRN2", num_swdge_queues=swdge_queues)

    x_d = nc.dram_tensor("x", [B, N, D], F32, kind="ExternalInput")
    w_d = nc.dram_tensor("w", [N, K, D, O], F32, kind="ExternalInput")
    c_d = nc.dram_tensor("consts", [P, CF], F32, kind="ExternalInput")
    out_d = nc.dram_tensor("out", [B, K, O], F32, kind="ExternalOutput")
    # scratch for broadcasting the per-iteration output back to SBUF layouts;
    # stored (o,k) to match H's free-dim order
    scr = nc.dram_tensor("scr", [B, O, K], F32, kind="Internal")

    with tile.TileContext(nc) as tc, nc.allow_low_precision(
            reason="bf16/f32r capsule routing, validated vs fp32 reference"):
        for _ in range(reps):
            _capsule(tc, x_d, w_d, c_d, out_d, scr, n_sweeps, wt_queue,
                     chunk or CHUNK)

    # TRN2 walrus codegen only allows one sync wait per instruction; these
    # Bacc passes split multi-wait instructions via event semaphores.
    import bass_rust as _bass_rust
    _bass_rust.move_matmul_waits_to_ldweights(nc.m)
    _bass_rust.generate_event_semaphores(nc)
    return nc


def _capsule(tc, x_d, w_d, c_d, out_d, scr, n_sweeps=NUM_ROUTINGS - 1,
             wt_queue="gpsimd", chunk=CHUNK):
    nc = tc.nc

    from contextlib import ExitStack
    ctx = ExitStack()
    consts = ctx.enter_context(tc.tile_pool(name="consts", bufs=1))
    hpool = ctx.enter_context(tc.tile_pool(name="hpool", bufs=1))
    wnatp = ctx.enter_context(tc.tile_pool(name="wnatp", bufs=2))
    wpermp = ctx.enter_context(tc.tile_pool(name="wpermp", bufs=2))
    wtp = ctx.enter_context(tc.tile_pool(name="wtp", bufs=2))
    ltp = ctx.enter_context(tc.tile_pool(name="ltp", bufs=2))
    small = ctx.enter_context(tc.tile_pool(name="small", bufs=2))
    sweep = ctx.enter_context(tc.tile_pool(name="sweep", bufs=2))
    prodp = ctx.enter_context(tc.tile_pool(name="prodp", bufs=1))
    psum_e = ctx.enter_context(tc.tile_pool(name="psum_e", bufs=2, space="PSUM"))
    psum_s = ctx.enter_context(tc.tile_pool(name="psum_s", bufs=1, space="PSUM"))
    psum_r = ctx.enter_context(tc.tile_pool(name="psum_r", bufs=1, space="PSUM"))
    psum_n = ctx.enter_context(tc.tile_pool(name="psum_n", bufs=1, space="PSUM"))

    # ---------------- constants (host-precomputed, one DMA) ----------------
    CON = consts.tile([P, CF], F32)
    nc.sync.dma_start(CON, c_d[:])
    ident = CON[:, ds(C_IDENT, P)]
    M = CON[:, ds(C_M, B * V)].rearrange("p (b v) -> p b v", b=B)
    Ib = CON[:, ds(C_IB, B)]
    ONESB16 = CON[:, ds(C_IB16, B)]
    MKT = CON[:, ds(C_MK, KO)]

    # ---------------- load + transpose x ----------------
    # x flat index = b*16384 + n*8 + d with n = nb*128 + gi*16 + v:
    #   X1[p=(b, nb), f=(gi, v, d)]
    XT2 = consts.tile([P, NB, GL, B], F32)  # XT2[(v,d), nb, gi, b]
    with tc.tile_pool(name="xprep", bufs=1) as xprep, \
         tc.tile_pool(name="psum_t", bufs=1, space="PSUM") as psum_t:
        X1 = xprep.tile([P, GL, V * D], F32)
        nc.sync.dma_start(X1, x_d.rearrange("b (nb gi v) d -> (b nb) gi (v d)",
                                            nb=NB, gi=GL, v=V))
        for gi in range(GL):
            pt = psum_t.tile([P, P], F32)
            nc.tensor.transpose(pt, X1[:, gi], ident)
            nc.vector.tensor_copy(
                XT2[:, :, gi, :], pt.rearrange("p (b nb) -> p nb b", b=B))

    # ---------------- einsum: H[(b,v'), (g,o,k)] bf16 ----------------
    H = hpool.tile([P, G, KO], BF16)
    ps0 = psum_s.tile([P, KO], F32, name="ps0")  # sum_g hats accumulator

    for nb in range(NB):
        # natural-layout W batch: 1MB contiguous
        wnat = wnatp.tile([P, K * D * O], F32, tag="wnat")
        nc.sync.dma_start(
            wnat, w_d[ds(nb * P, P)].rearrange("n k d o -> n (k d o)"))
        # permute (k,d,o)->(d,o,k) on-chip; output f32r (rounds for the PE)
        wperm = wpermp.tile([P, D, O, K], BF16, tag="wperm")
        nc.scalar.activation(
            wperm, wnat.rearrange("n (k d o) -> n d o k", k=K, d=D),
            ACT_F.Copy)
        # SBUF->SBUF reshuffle into matmul layout [(v,d),(o,k)] per group
        wt = wtp.tile([P, GL, KO], BF16, tag="wt")
        for gi in range(GL):
            if wt_queue == "gpsimd":
                weng = nc.gpsimd
            elif wt_queue == "sync":
                weng = nc.sync
            else:  # split across HWDGE + SWDGE rings
                weng = nc.sync if gi % 2 == 0 else nc.gpsimd
            weng.dma_start(
                wt[:, gi],
                wperm[ds(gi * V, V)].rearrange("v d o k -> v d (o k)"))
        # block-diagonal lhsT for the batch (f32r)
        LT = ltp.tile([P, GL, B, V], BF16, tag="LT")
        nc.vector.tensor_tensor(
            LT,
            XT2[:, nb][:, :, :, None].to_broadcast((P, GL, B, V)),
            M[:, None].to_broadcast((P, GL, B, V)),
            op=OP.mult)
        for gp in range(GL // 4):
            pe = psum_e.tile([P, 4, KO], F32, tag="pe")
            for part in range(4):
                gi = gp * 4 + part
                g = nb * GL + gi
                lhs = LT[:, gi].rearrange("p b v -> p (b v)")
                nc.tensor.matmul(pe[:, part], lhsT=lhs, rhs=wt[:, gi],
                                 start=True, stop=True)
                # s0 accumulator: sum_g hats in a second psum bank
                nc.tensor.matmul(ps0, lhsT=lhs, rhs=wt[:, gi],
                                 start=(g == 0), stop=(g == G - 1))
            g0 = nb * GL + gp * 4
            if (nb * 2 + gp) % 3 != 2:
                nc.vector.tensor_copy(H[:, ds(g0, 4)], pe)
            else:
                nc.scalar.activation(H[:, ds(g0, 4)], pe, ACT_F.Copy)

    # ---------------- s0 tail: mean over v', squash ----------------
    s0red = small.tile([P, KO], F32, tag="s0red")
    nc.vector.tensor_copy(s0red, ps0)
    ps0b = psum_r.tile([P, KO], F32, tag="r")
    nc.tensor.matmul(ps0b[:B], lhsT=ONESB16, rhs=s0red, start=True, stop=True)
    out0 = small.tile([B, KO], F32, tag="out0")
    _squash_bk(nc, small, out0, ps0b[:B])
    nc.sync.dma_start(scr.rearrange("b o k -> b (o k)"), out0)

    # ---------------- bias + sweeps ----------------
    bias = hpool.tile([P, G, K], F32)
    nc.vector.memset(bias, 0.0)

    NCH = G // chunk
    if n_sweeps == 0:
        # timing-bisection path only: layout within the 256 is (o,k), not
        # the reference (k,o) — not used for correctness runs
        nc.sync.dma_start(out_d.rearrange("b k o -> b (k o)"), out0)
    for it in range(n_sweeps):
        last = it == n_sweeps - 1
        # OutB[(b,v'), (o,k)] = out_it[b, o, k] broadcast over v' (bf16)
        OutBf = sweep.tile([P, KO], F32, tag="OutBf")
        nc.sync.dma_start(OutBf, scr[:, None].to_broadcast((B, V, O, K)))
        OutB = sweep.tile([P, O, K], BF16, tag="OutB")
        nc.vector.tensor_copy(OutB, OutBf.rearrange("p (o k) -> p o k", o=O))

        pr1 = psum_r.tile([P, KO], F32, tag="r", name=f"pr1_{it}")
        for j in range(NCH):
            gsl = ds(j * chunk, chunk)
            # R2: prod = H*OutB (packed bf16), then in-place tree over o
            prod = prodp.tile([P, chunk, O, K], BF16, tag="prod")
            nc.vector.tensor_tensor(
                prod, H[:, gsl].rearrange("p g (o k) -> p g o k", o=O),
                OutB[:, None].to_broadcast((P, chunk, O, K)),
                op=OP.mult)
            nc.vector.tensor_tensor(prod[:, :, 0:8], prod[:, :, 0:8],
                                    prod[:, :, 8:16], op=OP.add)
            nc.vector.tensor_tensor(prod[:, :, 0:4], prod[:, :, 0:4],
                                    prod[:, :, 4:8], op=OP.add)
            nc.vector.tensor_tensor(prod[:, :, 0:2], prod[:, :, 0:2],
                                    prod[:, :, 2:4], op=OP.add)
            ach = sweep.tile([P, chunk, K], BF16, tag="ach")
            nc.vector.tensor_tensor(ach, prod[:, :, 0], prod[:, :, 1],
                                    op=OP.add)
            nc.vector.tensor_tensor(bias[:, gsl], bias[:, gsl], ach, op=OP.add)
            # softmax over k
            expb = sweep.tile([P, chunk, K], BF16, tag="expb")
            nc.scalar.activation(expb, bias[:, gsl], ACT_F.Exp)
            den = sweep.tile([P, chunk], F32, tag="den")
            nc.vector.tensor_reduce(den, expb, axis=AX.X, op=OP.add)
            rden = sweep.tile([P, chunk], F32, tag="rden")
            nc.vector.reciprocal(rden, den)
            cch = sweep.tile([P, chunk, K], BF16, tag="cch")
            nc.vector.tensor_tensor(
                cch, expb, rden[:, :, None].to_broadcast((P, chunk, K)),
                op=OP.mult)
            # c-lhsT[(b,v'), (g, b', k')] = c * delta_{b,b'} (bf16, fast
            # tensor_scalar path: per-partition scalar operands are exempt
            # from the 2-byte packing rule)
            LTc = sweep.tile([P, chunk, B, K], BF16, tag="LTc")
            for b in range(B):
                nc.vector.tensor_scalar(LTc[:, :, b], cch, Ib[:, b:b + 1], None,
                                        op0=OP.mult)
            for i in range(chunk):
                g = j * chunk + i
                nc.tensor.matmul(pr1, lhsT=LTc[:, i].rearrange("p b k -> p (b k)"),
                                 rhs=H[:, g],
                                 start=(g == 0), stop=(g == G - 1))

        # diagonal extract: s[(b,k'), o] = sum_k pr1 * delta_{k,k'}
        prodD = small.tile([P, KO], F32, tag="prodD")
        nc.vector.tensor_tensor(prodD, pr1, MKT, op=OP.mult)
        sD = small.tile([P, O], F32, tag="sD")
        nc.vector.tensor_reduce(
            sD, prodD.rearrange("p (o k) -> p o k", o=O), axis=AX.X, op=OP.add)
        outN = small.tile([P, O], F32, tag="outN")
        _squash(nc, small, outN, sD, P)
        # NB: rearrange must live on the DRAM side — an SBUF source AP that
        # splits the partition dim silently reads the wrong partitions.
        if last:
            nc.sync.dma_start(out_d.rearrange("b k o -> (b k) o"), outN)
        else:
            # scr is (b, o, k) but outN partitions are (b,k') with o free;
            # (b,k) and o aren't adjacent in scr, so PE-transpose outN to
            # [o, (b,k)] first, then write with a 3-dim DRAM-side AP.
            ptN = psum_n.tile([O, P], F32, tag="ptN")
            nc.tensor.transpose(ptN, outN, ident)
            outT = small.tile([O, P], F32, tag="outT")
            nc.vector.tensor_copy(outT, ptN)
            nc.sync.dma_start(scr.rearrange("b o k -> o b k"), outT)

    ctx.close()


def _squash_bk(nc, pool, out, s_ap):
    """Per-k squash of s_ap [B, (o,k)]: norm over o only, for each k."""
    s_sb = pool.tile([B, O, K], F32, tag="sqk_s")
    nc.vector.tensor_copy(s_sb, s_ap.rearrange("b (o k) -> b o k", o=O))
    sq = pool.tile([B, O, K], F32, tag="sqk_tmp")
    nc.vector.tensor_tensor(sq, s_sb, s_sb, op=OP.mult)
    ss = pool.tile([B, K], F32, tag="sqk_ss")
    nc.vector.tensor_reduce(ss, sq.rearrange("b o k -> b k o"), axis=AX.X,
                            op=OP.add)
    rt = pool.tile([B, K], F32, tag="sqk_rt")
    nc.scalar.activation(rt, ss, ACT_F.Sqrt)
    dn = pool.tile([B, K], F32, tag="sqk_dn")
    nc.vector.tensor_scalar(dn, ss, 1.0, None, op0=OP.add)
    rc = pool.tile([B, K], F32, tag="sqk_rc")
    nc.vector.reciprocal(rc, dn)
    sc = pool.tile([B, K], F32, tag="sqk_sc")
    nc.vector.tensor_tensor(sc, rt, rc, op=OP.mult)
    nc.vector.tensor_tensor(
        out.rearrange("b (o k) -> b o k", o=O), s_sb,
        sc[:, None, :].to_broadcast((B, O, K)), op=OP.mult)


def _squash(nc, pool, out, s_ap, nparts):
    """out = s * sqrt(ss)/(1+ss), ss = sum over free dim of s^2 (per partition).

    s_ap must be in SBUF.
    (tensor_tensor_reduce hits an "ISA wrong length" codegen bug here,
    so square and reduce separately)"""
    sq = pool.tile([nparts, s_ap.shape[-1]], F32, tag="sq_tmp")
    nc.vector.tensor_tensor(sq, s_ap, s_ap, op=OP.mult)
    ss = pool.tile([nparts, 1], F32, tag="sq_ss")
    nc.vector.tensor_reduce(ss, sq, axis=AX.X, op=OP.add)
    rt = pool.tile([nparts, 1], F32, tag="sq_rt")
    nc.scalar.activation(rt, ss, ACT_F.Sqrt)
    dn = pool.tile([nparts, 1], F32, tag="sq_dn")
    nc.vector.tensor_scalar(dn, ss, 1.0, None, op0=OP.add)
    rc = pool.tile([nparts, 1], F32, tag="sq_rc")
    nc.vector.reciprocal(rc, dn)
    sc = pool.tile([nparts, 1], F32, tag="sq_sc")
    nc.vector.tensor_tensor(sc, rt, rc, op=OP.mult)
    nc.vector.tensor_scalar(out, s_ap, sc, None, op0=OP.mult)


_NC_CACHE = None


def kernel(x: np.ndarray, W: np.ndarray) -> np.ndarray:
    global _NC_CACHE
    x = np.ascontiguousarray(x, dtype=np.float32)
    W = np.ascontiguousarray(W, dtype=np.float32)
    if _NC_CACHE is None:
        _NC_CACHE = build_kernel()
    nc = _NC_CACHE
    n_cores = 8
    bsz = x.shape[0] // n_cores  # 8
    consts = _build_consts()
    in_maps = [{"x": x[c * bsz:(c + 1) * bsz], "w": W, "consts": consts}
               for c in range(n_cores)]
    res = run_bass_kernel_spmd(nc, in_maps, core_ids=list(range(n_cores)))
    return np.concatenate([r["out"] for r in res.results], axis=0)


# revision 31
# speedup vs baseline: 1.2227x; 1.2227x over previous
"""Trainium2 Bass kernel for CapsuleBlock (dynamic routing).

Reference computation:
  hats[b,n,k,o] = sum_d x[b,n,d] * W[n,k,d,o]       x:[64,2048,8] W:[2048,16,8,16]
  3 routing iterations (softmax over k, weighted sum over n, squash over o)
  output: [64, 16, 16]

Sharding: data-parallel over batch B=64 across 8 cores (B_local=8), W replicated.

Per-core layout notation: n = g*16 + v  (g in [0,128) "group", v in [0,16)),
d in [0,8), k in [0,16) out-capsule, o in [0,16) out-dim.

Key trick: einsum runs on the tensor engine with a block-diagonal lhsT
  lhsT[(v,d), (b,v')] = x[b, g*16+v', d] * delta_{v,v'}
so one 128-wide matmul per group computes hats for 16 n's x 8 b's at once:
  psum[(b,v'), (o,k)] = sum_{(v,d)} lhsT * W[g*16+v, k, d, o]
H lives in SBUF as [p=(b,v'), f=(g, o, k)] in bf16 (8MB). The free dim is
(o,k) — o outer — so the routing's sum-over-o is a 4-level in-place tree of
packed bf16 adds (DVE tensor_reduce gets no 16-bit speedup; packed adds do).

W streaming: W arrives [n,k,d,o]; the matmul needs [(v,d),(o,k)] per group.
Loading that layout directly from HBM costs 1024 DMAs of 64B runs (~10ms).
Instead: 16 x 1MB contiguous loads [n-batch, (k,d,o)], an on-chip permute
copy to [(n),(d,o,k)] in bf16 (1-cycle/col on the PE, and it halves the
reshuffle traffic), then one SBUF->SBUF DMA per group (SBUF-SBUF has no
small-run penalty) into [(v,d),(o,k)].

s0 (uniform-c iteration): gpsimd accumulates 8-group slabs of H as each W
batch completes (off the PE), then a small fp32 tree + ONESB16 matmul
averages over (g, v').

Routing iters 1,2 sweep H per 16-group chunk: bf16 product vs broadcast
OutB + o-tree -> ach; one batched softmax over all groups; c-lhsT built on
gpsimd; 16 accumulating bf16 matmuls per chunk into psum_R1[(b,k'), (o,k)];
diagonal extract (k==k' mask) -> squash -> out.

Toolchain workarounds (this container):
- walrus codegen allows only 1 sync wait/instruction -> run Bacc's
  move_matmul_waits_to_ldweights + generate_event_semaphores passes.
- gpsimd int shift/mod ops and DVE tensor_tensor_reduce hit "ISA wrong
  length" -> constant masks precomputed on host, squash unfused.
- DMA source APs that split the partition dim read wrong partitions ->
  keep rearranges on the DRAM side of every DMA.
"""

import numpy as np

import concourse.bass as bass
import concourse.mybir as mybir
import concourse.tile as tile
from concourse.bass import ds
from concourse.bass_utils import run_bass_kernel_spmd

F32 = mybir.dt.float32
F32R = mybir.dt.float32r
BF16 = mybir.dt.bfloat16
AX = mybir.AxisListType
OP = mybir.AluOpType
ACT_F = mybir.ActivationFunctionType

# per-core problem dims
B = 8        # local batch (64 / 8 cores)
N = 2048     # input capsules
K = 16       # output capsules
O = 16       # output capsule dim
D = 8        # input capsule dim
V = 16       # n's per group
G = N // V   # 128 groups
GL = 8       # groups per W batch
NB = G // GL  # 16 W batches (128 n's each)
P = 128
KO = K * O   # 256

NUM_ROUTINGS = 3
CHUNK = 32   # groups per sweep chunk


# host-precomputed constant masks, packed as one [128, CF] f32 input
# (gpsimd int shift/mod ops hit an "ISA wrong length" walrus codegen bug in
# this toolchain, so the masks cannot be built on-device)
C_IDENT = 0            # [128, 128] identity (PE transpose)
C_M = 128              # [128, B*V]  M[(v,d),(b,v')] = delta_{v,v'}
C_IB = C_M + B * V     # [128, B]    Ib[(b,v'),b'] = delta_{b,b'}
C_IB16 = C_IB + B      # [128, B]    Ib / 16
C_MK = C_IB16 + B      # [128, KO]   MKT[(b,k'),(o,k)] = delta_{k,k'}
CF = C_MK + KO


def _build_consts() -> np.ndarray:
    c = np.zeros((P, CF), dtype=np.float32)
    p = np.arange(P)
    c[:, C_IDENT:C_IDENT + P] = np.eye(P, dtype=np.float32)
    # M: partition p=(v,d) with v=p>>3 ; free f=(b,v') with v'=f%16
    f = np.arange(B * V)
    c[:, C_M:C_M + B * V] = ((p >> 3)[:, None] == (f % V)[None, :])
    # Ib: partition p=(b,v') with b=p>>4 ; free b'
    fb = np.arange(B)
    c[:, C_IB:C_IB + B] = ((p >> 4)[:, None] == fb[None, :])
    c[:, C_IB16:C_IB16 + B] = c[:, C_IB:C_IB + B] / V
    # MKT: partition p=(b,k') with k'=p%16 ; free f=(o,k) with k=f%16
    fk = np.arange(KO)
    c[:, C_MK:C_MK + KO] = ((p % K)[:, None] == (fk % K)[None, :])
    return c


def build_kernel(reps=1, n_sweeps=NUM_ROUTINGS - 1, swdge_queues=4,
                 wt_queue="sync", chunk=None, skip_s0_mm=False):
    nc = bass.Bass(trn_type="TRN2", num_swdge_queues=swdge_queues)

    x_d = nc.dram_tensor("x", [B, N, D], F32, kind="ExternalInput")
    w_d = nc.dram_tensor("w", [N, K, D, O], F32, kind="ExternalInput")
    c_d = nc.dram_tensor("consts", [P, CF], F32, kind="ExternalInput")
    out_d = nc.dram_tensor("out", [B, K, O], F32, kind="ExternalOutput")
    # scratch for broadcasting the per-iteration output back to SBUF layouts;
    # stored (o,k) to match H's free-dim order
    scr = nc.dram_tensor("scr", [B, O, K], F32, kind="Internal")

    with tile.TileContext(nc) as tc, nc.allow_low_precision(
            reason="bf16/f32r capsule routing, validated vs fp32 reference"):
        for _ in range(reps):
            _capsule(tc, x_d, w_d, c_d, out_d, scr, n_sweeps, wt_queue,
                     chunk or CHUNK, skip_s0_mm)

    # TRN2 walrus codegen only allows one sync wait per instruction; these
    # Bacc passes split multi-wait instructions via event semaphores.
    import bass_rust as _bass_rust
    _bass_rust.move_matmul_waits_to_ldweights(nc.m)
    _bass_rust.generate_event_semaphores(nc)
    return nc


def _capsule(tc, x_d, w_d, c_d, out_d, scr, n_sweeps=NUM_ROUTINGS - 1,
             wt_queue="gpsimd", chunk=CHUNK, skip_s0_mm=False):
    nc = tc.nc

    from contextlib import ExitStack
    ctx = ExitStack()
    consts = ctx.enter_context(tc.tile_pool(name="consts", bufs=1))
    hpool = ctx.enter_context(tc.tile_pool(name="hpool", bufs=1))
    wnatp = ctx.enter_context(tc.tile_pool(name="wnatp", bufs=2))
    wpermp = ctx.enter_context(tc.tile_pool(name="wpermp", bufs=2))
    wtp = ctx.enter_context(tc.tile_pool(name="wtp", bufs=2))
    ltp = ctx.enter_context(tc.tile_pool(name="ltp", bufs=2))
    small = ctx.enter_context(tc.tile_pool(name="small", bufs=2))
    sweep = ctx.enter_context(tc.tile_pool(name="sweep", bufs=2))
    prodp = ctx.enter_context(tc.tile_pool(name="prodp", bufs=1))
    psum_e = ctx.enter_context(tc.tile_pool(name="psum_e", bufs=2, space="PSUM"))
    psum_s = ctx.enter_context(tc.tile_pool(name="psum_s", bufs=1, space="PSUM"))
    psum_r = ctx.enter_context(tc.tile_pool(name="psum_r", bufs=1, space="PSUM"))
    psum_n = ctx.enter_context(tc.tile_pool(name="psum_n", bufs=1, space="PSUM"))

    # ---------------- constants (host-precomputed, one DMA) ----------------
    CON = consts.tile([P, CF], F32)
    nc.sync.dma_start(CON, c_d[:])
    ident = CON[:, ds(C_IDENT, P)]
    M = CON[:, ds(C_M, B * V)].rearrange("p (b v) -> p b v", b=B)
    Ib = CON[:, ds(C_IB, B)]
    ONESB16 = CON[:, ds(C_IB16, B)]
    MKT = CON[:, ds(C_MK, KO)]

    # ---------------- load + transpose x ----------------
    # x flat index = b*16384 + n*8 + d with n = nb*128 + gi*16 + v:
    #   X1[p=(b, nb), f=(gi, v, d)]
    XT2 = consts.tile([P, NB, GL, B], F32)  # XT2[(v,d), nb, gi, b]
    with tc.tile_pool(name="xprep", bufs=1) as xprep, \
         tc.tile_pool(name="psum_t", bufs=1, space="PSUM") as psum_t:
        X1 = xprep.tile([P, GL, V * D], F32)
        nc.sync.dma_start(X1, x_d.rearrange("b (nb gi v) d -> (b nb) gi (v d)",
                                            nb=NB, gi=GL, v=V))
        for gi in range(GL):
            pt = psum_t.tile([P, P], F32)
            nc.tensor.transpose(pt, X1[:, gi], ident)
            nc.vector.tensor_copy(
                XT2[:, :, gi, :], pt.rearrange("p (b nb) -> p nb b", b=B))

    # ---------------- einsum: H[(b,v'), (g,o,k)] bf16 ----------------
    H = hpool.tile([P, G, KO], BF16)
    ps0 = psum_s.tile([P, KO], F32, name="ps0")  # sum_g hats accumulator

    for nb in range(NB):
        # natural-layout W batch: 1MB contiguous
        wnat = wnatp.tile([P, K * D * O], F32, tag="wnat")
        nc.sync.dma_start(
            wnat, w_d[ds(nb * P, P)].rearrange("n k d o -> n (k d o)"))
        # permute (k,d,o)->(d,o,k) on-chip; output f32r (rounds for the PE)
        wperm = wpermp.tile([P, D, O, K], BF16, tag="wperm")
        nc.scalar.activation(
            wperm, wnat.rearrange("n (k d o) -> n d o k", k=K, d=D),
            ACT_F.Copy)
        # SBUF->SBUF reshuffle into matmul layout [(v,d),(o,k)] per group
        wt = wtp.tile([P, GL, KO], BF16, tag="wt")
        for gi in range(GL):
            if wt_queue == "gpsimd":
                weng = nc.gpsimd
            elif wt_queue == "sync":
                weng = nc.sync
            else:  # split across HWDGE + SWDGE rings
                weng = nc.sync if gi % 2 == 0 else nc.gpsimd
            weng.dma_start(
                wt[:, gi],
                wperm[ds(gi * V, V)].rearrange("v d o k -> v d (o k)"))
        # block-diagonal lhsT for the batch (f32r)
        LT = ltp.tile([P, GL, B, V], BF16, tag="LT")
        nc.vector.tensor_tensor(
            LT,
            XT2[:, nb][:, :, :, None].to_broadcast((P, GL, B, V)),
            M[:, None].to_broadcast((P, GL, B, V)),
            op=OP.mult)
        for gp in range(GL // 4):
            pe = psum_e.tile([P, 4, KO], F32, tag="pe")
            for part in range(4):
                gi = gp * 4 + part
                g = nb * GL + gi
                lhs = LT[:, gi].rearrange("p b v -> p (b v)")
                nc.tensor.matmul(pe[:, part], lhsT=lhs, rhs=wt[:, gi],
                                 start=True, stop=True)
                # s0 accumulator: sum_g hats in a second psum bank
                # (skip_s0_mm is a timing-diagnostic mode: s0 comes out
                # wrong; never used for correctness runs)
                if not skip_s0_mm or g == 0:
                    nc.tensor.matmul(ps0, lhsT=lhs, rhs=wt[:, gi],
                                     start=(g == 0), stop=(g == 0 and skip_s0_mm) or (g == G - 1))
            g0 = nb * GL + gp * 4
            if (nb * 2 + gp) % 3 != 2:
                nc.vector.tensor_copy(H[:, ds(g0, 4)], pe)
            else:
                nc.scalar.activation(H[:, ds(g0, 4)], pe, ACT_F.Copy)

    # ---------------- s0 tail: mean over v', squash ----------------
    s0red = small.tile([P, KO], F32, tag="s0red")
    nc.vector.tensor_copy(s0red, ps0)
    ps0b = psum_r.tile([P, KO], F32, tag="r")
    nc.tensor.matmul(ps0b[:B], lhsT=ONESB16, rhs=s0red, start=True, stop=True)
    out0 = small.tile([B, KO], F32, tag="out0")
    _squash_bk(nc, small, out0, ps0b[:B])
    nc.sync.dma_start(scr.rearrange("b o k -> b (o k)"), out0)

    # ---------------- bias + sweeps ----------------
    bias = hpool.tile([P, G, K], F32)
    nc.vector.memset(bias, 0.0)

    NCH = G // chunk
    if n_sweeps == 0:
        # timing-bisection path only: layout within the 256 is (o,k), not
        # the reference (k,o) — not used for correctness runs
        nc.sync.dma_start(out_d.rearrange("b k o -> b (k o)"), out0)
    for it in range(n_sweeps):
        last = it == n_sweeps - 1
        # OutB[(b,v'), (o,k)] = out_it[b, o, k] broadcast over v' (bf16)
        OutBf = sweep.tile([P, KO], F32, tag="OutBf")
        nc.sync.dma_start(OutBf, scr[:, None].to_broadcast((B, V, O, K)))
        OutB = sweep.tile([P, O, K], BF16, tag="OutB")
        nc.vector.tensor_copy(OutB, OutBf.rearrange("p (o k) -> p o k", o=O))

        pr1 = psum_r.tile([P, KO], F32, tag="r", name=f"pr1_{it}")
        for j in range(NCH):
            gsl = ds(j * chunk, chunk)
            # R2: prod = H*OutB (packed bf16), then in-place tree over o
            prod = prodp.tile([P, chunk, O, K], BF16, tag="prod")
            nc.vector.tensor_tensor(
                prod, H[:, gsl].rearrange("p g (o k) -> p g o k", o=O),
                OutB[:, None].to_broadcast((P, chunk, O, K)),
                op=OP.mult)
            nc.vector.tensor_tensor(prod[:, :, 0:8], prod[:, :, 0:8],
                                    prod[:, :, 8:16], op=OP.add)
            nc.vector.tensor_tensor(prod[:, :, 0:4], prod[:, :, 0:4],
                                    prod[:, :, 4:8], op=OP.add)
            nc.vector.tensor_tensor(prod[:, :, 0:2], prod[:, :, 0:2],
                                    prod[:, :, 2:4], op=OP.add)
            ach = sweep.tile([P, chunk, K], BF16, tag="ach")
            nc.vector.tensor_tensor(ach, prod[:, :, 0], prod[:, :, 1],
                                    op=OP.add)
            nc.vector.tensor_tensor(bias[:, gsl], bias[:, gsl], ach, op=OP.add)
            # softmax over k
            expb = sweep.tile([P, chunk, K], BF16, tag="expb")
            nc.scalar.activation(expb, bias[:, gsl], ACT_F.Exp)
            den = sweep.tile([P, chunk], F32, tag="den")
            nc.vector.tensor_reduce(den, expb, axis=AX.X, op=OP.add)
            rden = sweep.tile([P, chunk], F32, tag="rden")
            nc.vector.reciprocal(rden, den)
            cch = sweep.tile([P, chunk, K], BF16, tag="cch")
            nc.vector.tensor_tensor(
                cch, expb, rden[:, :, None].to_broadcast((P, chunk, K)),
                op=OP.mult)
            # c-lhsT[(b,v'), (g, b', k')] = c * delta_{b,b'} (bf16, fast
            # tensor_scalar path: per-partition scalar operands are exempt
            # from the 2-byte packing rule)
            LTc = sweep.tile([P, chunk, B, K], BF16, tag="LTc")
            for b in range(B):
                nc.vector.tensor_scalar(LTc[:, :, b], cch, Ib[:, b:b + 1], None,
                                        op0=OP.mult)
            for i in range(chunk):
                g = j * chunk + i
                nc.tensor.matmul(pr1, lhsT=LTc[:, i].rearrange("p b k -> p (b k)"),
                                 rhs=H[:, g],
                                 start=(g == 0), stop=(g == G - 1))

        # diagonal extract: s[(b,k'), o] = sum_k pr1 * delta_{k,k'}
        prodD = small.tile([P, KO], F32, tag="prodD")
        nc.vector.tensor_tensor(prodD, pr1, MKT, op=OP.mult)
        sD = small.tile([P, O], F32, tag="sD")
        nc.vector.tensor_reduce(
            sD, prodD.rearrange("p (o k) -> p o k", o=O), axis=AX.X, op=OP.add)
        outN = small.tile([P, O], F32, tag="outN")
        _squash(nc, small, outN, sD, P)
        # NB: rearrange must live on the DRAM side — an SBUF source AP that
        # splits the partition dim silently reads the wrong partitions.
        if last:
            nc.sync.dma_start(out_d.rearrange("b k o -> (b k) o"), outN)
        else:
            # scr is (b, o, k) but outN partitions are (b,k') with o free;
            # (b,k) and o aren't adjacent in scr, so PE-transpose outN to
            # [o, (b,k)] first, then write with a 3-dim DRAM-side AP.
            ptN = psum_n.tile([O, P], F32, tag="ptN")
            nc.tensor.transpose(ptN, outN, ident)
            outT = small.tile([O, P], F32, tag="outT")
            nc.vector.tensor_copy(outT, ptN)
            nc.sync.dma_start(scr.rearrange("b o k -> o b k"), outT)

    ctx.close()


def _squash_bk(nc, pool, out, s_ap):
    """Per-k squash of s_ap [B, (o,k)]: norm over o only, for each k."""
    s_sb = pool.tile([B, O, K], F32, tag="sqk_s")
    nc.vector.tensor_copy(s_sb, s_ap.rearrange("b (o k) -> b o k", o=O))
    sq = pool.tile([B, O, K], F32, tag="sqk_tmp")
    nc.vector.tensor_tensor(sq, s_sb, s_sb, op=OP.mult)
    ss = pool.tile([B, K], F32, tag="sqk_ss")
    nc.vector.tensor_reduce(ss, sq.rearrange("b o k -> b k o"), axis=AX.X,
                            op=OP.add)
    rt = pool.tile([B, K], F32, tag="sqk_rt")
    nc.scalar.activation(rt, ss, ACT_F.Sqrt)
    dn = pool.tile([B, K], F32, tag="sqk_dn")
    nc.vector.tensor_scalar(dn, ss, 1.0, None, op0=OP.add)
    rc = pool.tile([B, K], F32, tag="sqk_rc")
    nc.vector.reciprocal(rc, dn)
    sc = pool.tile([B, K], F32, tag="sqk_sc")
    nc.vector.tensor_tensor(sc, rt, rc, op=OP.mult)
    nc.vector.tensor_tensor(
        out.rearrange("b (o k) -> b o k", o=O), s_sb,
        sc[:, None, :].to_broadcast((B, O, K)), op=OP.mult)


def _squash(nc, pool, out, s_ap, nparts):
    """out = s * sqrt(ss)/(1+ss), ss = sum over free dim of s^2 (per partition).

    s_ap must be in SBUF.
    (tensor_tensor_reduce hits an "ISA wrong length" codegen bug here,
    so square and reduce separately)"""
    sq = pool.tile([nparts, s_ap.shape[-1]], F32, tag="sq_tmp")
    nc.vector.tensor_tensor(sq, s_ap, s_ap, op=OP.mult)
    ss = pool.tile([nparts, 1], F32, tag="sq_ss")
    nc.vector.tensor_reduce(ss, sq, axis=AX.X, op=OP.add)
    rt = pool.tile([nparts, 1], F32, tag="sq_rt")
    nc.scalar.activation(rt, ss, ACT_F.Sqrt)
    dn = pool.tile([nparts, 1], F32, tag="sq_dn")
    nc.vector.tensor_scalar(dn, ss, 1.0, None, op0=OP.add)
    rc = pool.tile([nparts, 1], F32, tag="sq_rc")
    nc.vector.reciprocal(rc, dn)
    sc = pool.tile([nparts, 1], F32, tag="sq_sc")
    nc.vector.tensor_tensor(sc, rt, rc, op=OP.mult)
    nc.vector.tensor_scalar(out, s_ap, sc, None, op0=OP.mult)


_NC_CACHE = None


def kernel(x: np.ndarray, W: np.ndarray) -> np.ndarray:
    global _NC_CACHE
    x = np.ascontiguousarray(x, dtype=np.float32)
    W = np.ascontiguousarray(W, dtype=np.float32)
    if _NC_CACHE is None:
        _NC_CACHE = build_kernel()
    nc = _NC_CACHE
    n_cores = 8
    bsz = x.shape[0] // n_cores  # 8
    consts = _build_consts()
    in_maps = [{"x": x[c * bsz:(c + 1) * bsz], "w": W, "consts": consts}
               for c in range(n_cores)]
    res = run_bass_kernel_spmd(nc, in_maps, core_ids=list(range(n_cores)))
    return np.concatenate([r["out"] for r in res.results], axis=0)
